# revision 51
# baseline (speedup 1.0000x reference)
"""Trainium2 Bass kernel for the EnhancedEncoderLayer (dense MHA + low-rank
top-k sparse attention + FFN, two layernorms).

Sharding: 8 cores = (batch b in 0..3) x (query-half h in {0,1}). Each core
computes output rows [b, h*512:(h+1)*512, :]. K/V-side projections are
computed redundantly per batch pair (no cross-core communication).

The host permutes src[b].T columns so each core's own query tokens are
columns 0..511 (attention contracts over all keys, so key order is
irrelevant); this keeps the SPMD program identical across cores.

v2 highlights vs the f32r baseline:
- whole trunk in bf16 (weights host-prepped into contiguous stream-order
  layouts -> trivial DMA descriptors, half the HBM traffic);
- v / v-sparse projections are x-stationary and written token-major
  directly (no PE transposes, no ACT copies);
- softmax exps processed as 2-PSUM-bank [128,1024] ACTs;
- top-k threshold bisection runs on bf16 scores, 13 iters, counts split
  between DVE (exact is_ge, qt 0,1) and the ACT engine (sign-sum, qt 2,3);
- ln1 gamma/beta folded into ff1 weights host-side; v/ff2 biases folded
  into the matmuls via augmented ones-row contraction chunks;
- tail is qt-pipelined: out_proj/spmm/fuse/LN1/xhat-transpose per query
  tile; ff2 runs nh-grouped with partial LN2 stats.
"""
import sys
import os
import contextlib

for _p in ('/opt/trn_rl_repo',):
    if _p not in sys.path:
        sys.path.insert(0, _p)

import numpy as np
import concourse.bacc as bacc
import concourse.tile as tile
from concourse import mybir
from concourse.bass_utils import run_bass_kernel_spmd
from concourse.masks import make_identity

F32 = mybir.dt.float32
BF16 = mybir.dt.bfloat16
AF = mybir.ActivationFunctionType
OP = mybir.AluOpType

B, S, D, H, R, DFF = 4, 1024, 1024, 16, 64, 4096
DH = D // H          # 64
SQ = S // 2          # 512 own queries per core
KK = max(1, int(S * 0.2))   # 204
KC = D // 128        # 8 contraction chunks over D
FC = DFF // 128      # 32 chunks over DFF
NQT = SQ // 128      # 4 query tiles
NTOK = S // 128      # 8 token tiles
NFT = KC             # 8 feature tiles of 128 over D
BISECT_ITERS = 13
INV_SQRT = 0.125     # 1/sqrt(DH) == 1/sqrt(R)

_cached = {}


def _build(zb=False):
    # zb: v/vsp, ff2 and out_proj biases are all exactly zero -- skip the
    # augmented bias chunks and the xot bias add entirely.
    nc = bacc.Bacc()
    NKV = KC if zb else KC + 1
    NF2 = FC if zb else FC + 1

    def din(name, shape, dt=F32):
        return nc.declare_dram_parameter(name, list(shape), dt, isOutput=False)

    xT = din("xT", [KC + 1, 128, S], BF16)   # [kc, p, s]; kc=8: ones row
    x_own = din("x_own", [SQ, D])            # own rows, token-major, f32
    wq = din("wq", [NFT, 128, KC, 128], BF16)
    wk = din("wk", [NFT, 128, KC, 128], BF16)
    wv = din("wv", [128, KC + 1, D], BF16)   # kc=8 row0: bias
    vp = din("vp", [128, KC + 1, D], BF16)
    wo = din("wo", [128, KC, D], BF16)
    qkp = din("qkp", [128, KC, 2 * R], BF16)  # cols 0:64 Qp, 64:128 Kp
    f1 = din("f1", [8, 128, KC, 512], BF16)
    f2 = din("f2", [FC + 1, 128, D], BF16)  # chunk FC row0: b2
    bias_cols = din("bias_cols", [128, 80])  # 0:24 qkv, 24:32 vp, 32:64 ff1,
    #                                          64:72 ln1_g, 72:80 ln1_b
    bqkp = din("bqkp", [64, 2])              # col0 Qp_b, col1 Kp_b
    # host-broadcast rows: a = (sig*bo, vb, vspb); b = (b2+be1, g1, g2, be2)
    bca_p = din("bca", [128, D])
    bcb_p = din("bcb", [128, 4, D])
    sig_col = din("sig_col", [128, 1])
    oms_col = din("oms_col", [128, 1])
    out = nc.declare_dram_parameter("out", [SQ, D], F32, isOutput=True)
    DBG = bool(os.environ.get("BASSK_DEBUG"))
    if DBG:
        dbg_fuse = nc.declare_dram_parameter("dbg_fuse", [SQ, D], F32,
                                             isOutput=True)
        dbg_lo = nc.declare_dram_parameter("dbg_lo", [128, NQT], F32,
                                           isOutput=True)
        dbg_rs = nc.declare_dram_parameter("dbg_rs", [128, NQT], F32,
                                           isOutput=True)
        dbg_psp = nc.declare_dram_parameter("dbg_psp", [128, S], BF16,
                                            isOutput=True)
        dbg_kT = nc.declare_dram_parameter("dbg_kT", [128, KC * S], BF16,
                                           isOutput=True)
        dbg_qT = nc.declare_dram_parameter("dbg_qT", [128, KC * SQ], BF16,
                                           isOutput=True)
        dbg_ctx = nc.declare_dram_parameter("dbg_ctx", [128, KC * SQ], BF16,
                                            isOutput=True)
        dbg_vaug = nc.declare_dram_parameter("dbg_vaug",
                                             [128, NTOK * H * (DH + 1)],
                                             BF16, isOutput=True)

    with tile.TileContext(nc) as tc:
        est = contextlib.ExitStack()
        with est:
            # ---------------- constants ----------------
            consts = est.enter_context(tc.tile_pool(name="consts", bufs=1))

            ident_f = consts.tile([128, 128], F32, name="ident_f")
            make_identity(nc, ident_f)
            ident_b = consts.tile([128, 128], BF16, name="ident_b")
            nc.vector.tensor_copy(out=ident_b, in_=ident_f)

            eps_t = consts.tile([128, 1], F32, name="eps_t")
            nc.vector.memset(eps_t, 1e-5)
            ones_b = consts.tile([128, 1], BF16, name="ones_b")
            nc.vector.memset(ones_b, 1.0)
            ones1 = consts.tile([128, 1], F32, name="ones1")
            nc.vector.memset(ones1, 1.0)
            ones16 = consts.tile([128, 16], BF16, name="ones16")
            nc.vector.memset(ones16, 1.0)

            sig_bc = consts.tile([128, 1], F32, name="sig_bc")
            nc.gpsimd.dma_start(out=sig_bc, in_=sig_col.ap())
            oms_bc = consts.tile([128, 1], F32, name="oms_bc")
            nc.gpsimd.dma_start(out=oms_bc, in_=oms_col.ap())

            bc = consts.tile([128, 80], F32, name="bc")
            nc.gpsimd.dma_start(out=bc, in_=bias_cols.ap())
            bqkv_c = bc[:, 0:24]
            b1_c = bc[:, 32:64]
            g1_c = bc[:, 64:72]
            be1_c = bc[:, 72:80]
            bqkp_t = consts.tile([64, 2], F32, name="bqkp_t")
            nc.gpsimd.dma_start(out=bqkp_t, in_=bqkp.ap())
            bqp_c = bqkp_t[:, 0:1]
            bkp_c = bqkp_t[:, 1:2]

            # host-broadcast rows needed during the attention window
            bo_sig = consts.tile([128, D], F32, name="bo_sig")
            nc.gpsimd.dma_start(out=bo_sig, in_=bca_p.ap())

            # own-token residual (+ sig*bo) -- loaded later, used in tail
            xot_pool = est.enter_context(tc.tile_pool(name="xot_pool",
                                                      bufs=1))
            xot = xot_pool.tile([128, NQT, D], F32, name="xot")

            def load_xot():
                for qt in range(NQT):
                    nc.gpsimd.dma_start(
                        out=xot[:, qt, :],
                        in_=x_own.ap()[qt * 128:qt * 128 + 128, :])
                    if not zb:
                        nc.gpsimd.tensor_add(xot[:, qt, :], xot[:, qt, :],
                                             bo_sig)

            # bisect state
            bis = est.enter_context(tc.tile_pool(name="bis", bufs=1))
            lo = bis.tile([128, NQT], F32, name="lo")
            hi = bis.tile([128, NQT], F32, name="hi")
            mid = bis.tile([128, NQT], F32, name="mid")
            cnts = bis.tile([128, NQT], F32, name="cnts")
            pred = bis.tile([128, NQT], mybir.dt.uint32, name="pred")
            rs_sp = bis.tile([128, NQT], F32, name="rs_sp")
            rcp_sp = bis.tile([128, NQT], F32, name="rcp_sp")
            scr_d = bis.tile([128, S], BF16, name="scr_d")
            scr_p = bis.tile([128, S], BF16, name="scr_p")
            nmid = bis.tile([128, NQT], F32, name="nmid")
            thr = bis.tile([128, NQT], F32, name="thr")
            # exact counts for qt 0,1 (DVE is_ge); sign-sum counts for qt 2,3
            # (ACT): sum(sign(x - mid)) >= 2*KK - S  <=>  count(>=mid) >= KK
            nc.vector.memset(thr[:, 0:2], float(KK))
            nc.vector.memset(thr[:, 2:4], float(2 * KK - S))

            # long-lived activation tiles
            sp_pool = est.enter_context(tc.tile_pool(name="sp_pool", bufs=1))
            Vsp = sp_pool.tile([128, NTOK, D], BF16, name="Vsp")  # tok-major
            kspT = sp_pool.tile([64, S], BF16, name="kspT")
            qspT = sp_pool.tile([64, SQ], BF16, name="qspT")

            dn_stack = contextlib.ExitStack()
            dn_pool = dn_stack.enter_context(
                tc.tile_pool(name="dn_pool", bufs=1))
            kT = dn_pool.tile([128, KC, S], BF16, name="kT")
            Vaug = dn_pool.tile([128, NTOK, H * (DH + 1)], BF16, name="Vaug")
            qT = dn_pool.tile([128, KC, SQ], BF16, name="qT")

            Vaug_h = Vaug.rearrange("p t (h c) -> p t h c", c=DH + 1)
            for t in range(NTOK):
                nc.gpsimd.tensor_copy(out=Vaug_h[:, t, :, DH:DH + 1],
                                      in_=ones16)

            xln_stack = contextlib.ExitStack()
            xln_pool = xln_stack.enter_context(
                tc.tile_pool(name="xln_pool", bufs=1, side="right"))
            xlnT = xln_pool.tile([128, KC, SQ], BF16, name="xlnT")

            psp_stack = contextlib.ExitStack()
            psp_pool = psp_stack.enter_context(
                tc.tile_pool(name="psp_pool", bufs=1, side="right"))
            psp = [psp_pool.tile([128, S], BF16, name=f"psp{qt}")
                   for qt in range(NQT)]
            ctx_stack = contextlib.ExitStack()
            ctx_pool = ctx_stack.enter_context(
                tc.tile_pool(name="ctx_pool", bufs=1, side="right"))
            ctxT = ctx_pool.tile([128, KC, SQ], BF16, name="ctxT")

            # out_proj weights (resident through the tail)
            wo_stack = contextlib.ExitStack()
            wo_pool = wo_stack.enter_context(
                tc.tile_pool(name="wo_pool", bufs=1, side="right"))
            wo_t = wo_pool.tile([128, KC, D], BF16, name="wo_t")

            # ============ projections + sparse path + attention ============
            with contextlib.ExitStack() as ph0:
                xt_pool = ph0.enter_context(
                    tc.tile_pool(name="xt_pool", bufs=1))
                wres = ph0.enter_context(tc.tile_pool(name="wres", bufs=1))
                wstr = ph0.enter_context(tc.tile_pool(name="wstr", bufs=3))
                pt_pool = ph0.enter_context(
                    tc.tile_pool(name="pt_pool", bufs=6))
                rc_pool = ph0.enter_context(
                    tc.tile_pool(name="rc_pool", bufs=3))
                ps_small = ph0.enter_context(
                    tc.tile_pool(name="ps_small", bufs=2, space="PSUM"))
                ps_big = ph0.enter_context(
                    tc.tile_pool(name="ps_big", bufs=2, space="PSUM"))
                ps_ctx = ph0.enter_context(
                    tc.tile_pool(name="ps_ctx", bufs=2, space="PSUM"))

                # small sparse weights first, then xT on both queues
                qkpt = wstr.tile([128, KC, 2 * R], BF16, name="qkpt",
                                 tag="wt")
                nc.sync.dma_start(out=qkpt, in_=qkp.ap())
                qeng = [nc.scalar, nc.sync, nc.gpsimd]
                xTt = xt_pool.tile([128, KC + 1, S], BF16, name="xTt")
                wv_h = wres.tile([128, KC + 1, D], BF16, name="wv_t",
                                 tag="wv")
                vp_h = wres.tile([128, KC + 1, D], BF16, name="vp_t",
                                 tag="vp")
                for kc in range(NKV):
                    eng = nc.scalar if kc % 2 == 0 else nc.sync
                    eng.dma_start(out=xTt[:, kc, :], in_=xT.ap()[kc])
                for kc in range(NKV):
                    qeng[(kc + 1) % 3].dma_start(out=wv_h[:, kc, :],
                                                 in_=wv.ap()[:, kc, :])
                    qeng[(kc + 2) % 3].dma_start(out=vp_h[:, kc, :],
                                                 in_=vp.ap()[:, kc, :])
                nc.gpsimd.dma_start(out=wo_t[:, 0:4, :],
                                    in_=wo.ap()[:, 0:4, :])
                nc.gpsimd.dma_start(out=wo_t[:, 4:8, :],
                                    in_=wo.ap()[:, 4:8, :])

                # ---- sparse projections ----
                qpt = qkpt[:, :, 0:R]
                kpt = qkpt[:, :, R:2 * R]
                with nc.named_scope("p0_ksp_qsp"):
                    for nh in range(2):
                        ps = ps_small.tile([128, 512], F32, name="ps",
                                           tag="ps")
                        for kc in range(KC):
                            nc.tensor.matmul(
                                ps[0:64, :], kpt[:, kc, :],
                                xTt[:, kc, nh * 512:nh * 512 + 512],
                                start=(kc == 0), stop=(kc == KC - 1))
                        nc.vector.tensor_scalar(
                            out=kspT[:, nh * 512:nh * 512 + 512],
                            in0=ps[0:64, :], scalar1=bkp_c, scalar2=None,
                            op0=OP.add)
                    ps = ps_small.tile([128, 512], F32, name="ps", tag="ps")
                    for kc in range(KC):
                        nc.tensor.matmul(ps[0:64, :], qpt[:, kc, :],
                                         xTt[:, kc, 0:SQ],
                                         start=(kc == 0), stop=(kc == KC - 1))
                    nc.vector.tensor_scalar(
                        out=qspT, in0=ps[0:64, :], scalar1=bqp_c,
                        scalar2=None, op0=OP.add)

                with nc.named_scope("p2_ssp"):
                    for qt in range(NQT):
                        ps = ps_big.tile([128, S], F32, name="ps2", tag="ps2")
                        for nh in range(2):
                            nc.tensor.matmul(
                                ps[:, nh * 512:nh * 512 + 512],
                                qspT[:, qt * 128:qt * 128 + 128],
                                kspT[:, nh * 512:nh * 512 + 512],
                                start=True, stop=True)
                        nc.scalar.activation(out=psp[qt], in_=ps,
                                             func=AF.Exp, scale=INV_SQRT)

                # ---- top-k threshold bisection (DVE qt 0,1 / Pool qt 2,3) --
                with nc.named_scope("p3_bisect"):
                    nc.vector.memset(lo, 0.0)
                    nc.vector.memset(hi, 16.0)
                    for it in range(BISECT_ITERS):
                        nc.vector.tensor_add(mid, lo, hi)
                        nc.vector.tensor_scalar_mul(mid, mid, 0.5)
                        nc.vector.tensor_scalar_mul(nmid, mid, -1.0)
                        for qt in range(2):
                            nc.vector.scalar_tensor_tensor(
                                out=scr_d, in0=psp[qt],
                                scalar=mid[:, qt:qt + 1],
                                in1=ones_b.to_broadcast([128, S]),
                                op0=OP.is_ge, op1=OP.mult,
                                accum_out=cnts[:, qt:qt + 1])
                        for qt in range(2, NQT):
                            nc.scalar.activation(
                                out=scr_p, in_=psp[qt], func=AF.Sign,
                                bias=nmid[:, qt:qt + 1], scale=1.0,
                                accum_out=cnts[:, qt:qt + 1])
                        nc.vector.tensor_tensor(out=pred, in0=cnts, in1=thr,
                                                op=OP.is_ge)
                        nc.vector.copy_predicated(lo, pred, mid)
                        nc.vector.tensor_tensor(out=pred, in0=cnts, in1=thr,
                                                op=OP.is_lt)
                        nc.vector.copy_predicated(hi, pred, mid)
                    for qt in range(NQT):
                        eng = nc.vector
                        eng.scalar_tensor_tensor(
                            out=psp[qt], in0=psp[qt],
                            scalar=lo[:, qt:qt + 1],
                            in1=psp[qt], op0=OP.is_ge, op1=OP.mult,
                            accum_out=rs_sp[:, qt:qt + 1])
                    if DBG:
                        nc.sync.dma_start(out=dbg_lo.ap(), in_=lo)
                        nc.sync.dma_start(out=dbg_rs.ap(), in_=rs_sp)
                        nc.sync.dma_start(out=dbg_psp.ap(), in_=psp[0])
                    nc.vector.tensor_scalar(out=rs_sp, in0=rs_sp,
                                            scalar1=1e-9, scalar2=None,
                                            op0=OP.add)
                    nc.vector.reciprocal(rcp_sp, rs_sp)
                    nc.vector.tensor_scalar_mul(rcp_sp, rcp_sp, oms_bc)

                # ---- interleaved kq proj + v/vsp proj + dense attention ----
                def kq_proj(ft):
                    wkc = wstr.tile([128, KC, 128], BF16, name="wt",
                                    tag="wt")
                    eng = nc.scalar if ft % 2 == 0 else nc.sync
                    eng.dma_start(out=wkc, in_=wk.ap()[ft])
                    for nh in range(2):
                        ps = ps_small.tile([128, 512], F32, name="ps",
                                           tag="ps")
                        for kc in range(KC):
                            nc.tensor.matmul(
                                ps, wkc[:, kc, :],
                                xTt[:, kc, nh * 512:nh * 512 + 512],
                                start=(kc == 0), stop=(kc == KC - 1))
                        nc.vector.tensor_scalar(
                            out=kT[:, ft, nh * 512:nh * 512 + 512],
                            in0=ps, scalar1=bqkv_c[:, 8 + ft:8 + ft + 1],
                            scalar2=None, op0=OP.add)
                    wqc = wstr.tile([128, KC, 128], BF16, name="wt",
                                    tag="wt")
                    eng.dma_start(out=wqc, in_=wq.ap()[ft])
                    ps = ps_small.tile([128, 512], F32, name="ps", tag="ps")
                    for kc in range(KC):
                        nc.tensor.matmul(ps, wqc[:, kc, :],
                                         xTt[:, kc, 0:SQ],
                                         start=(kc == 0), stop=(kc == KC - 1))
                    nc.vector.tensor_scalar(
                        out=qT[:, ft, :], in0=ps,
                        scalar1=bqkv_c[:, ft:ft + 1], scalar2=None,
                        op0=OP.add)

                def v_proj(t):
                    # x-stationary: out token-major [tok128, D]; bias folded
                    # into the 9th contraction chunk (ones row x bias row)
                    for path in range(2):  # 0: dense v -> Vaug, 1: vsp -> Vsp
                        w_h = wv_h if path == 0 else vp_h
                        psA = ps_small.tile([128, 512], F32, name="ps",
                                            tag="ps")
                        psB = ps_small.tile([128, 512], F32, name="ps",
                                            tag="ps")
                        for kc in range(NKV):
                            lhs = xTt[:, kc, t * 128:t * 128 + 128]
                            nc.tensor.matmul(psA, lhs, w_h[:, kc, 0:512],
                                             start=(kc == 0),
                                             stop=(kc == NKV - 1))
                            nc.tensor.matmul(psB, lhs, w_h[:, kc, 512:D],
                                             start=(kc == 0),
                                             stop=(kc == NKV - 1))
                        if path == 0:
                            pa = psA.rearrange("p (h c) -> p h c", c=DH)
                            pb = psB.rearrange("p (h c) -> p h c", c=DH)
                            nc.scalar.copy(
                                out=Vaug_h[:, t, 0:8, 0:DH], in_=pa)
                            nc.scalar.copy(
                                out=Vaug_h[:, t, 8:16, 0:DH], in_=pb)
                        else:
                            nc.scalar.copy(out=Vsp[:, t, 0:512], in_=psA)
                            nc.scalar.copy(out=Vsp[:, t, 512:D], in_=psB)

                def attn_head(hh):
                    po = 64 * (hh % 2)
                    ft = hh // 2
                    pts = []
                    for tp in range(4):
                        ps = ps_big.tile([128, S], F32, name="ps_s",
                                         tag="ps2")
                        for half in range(2):
                            t = 2 * tp + half
                            nc.tensor.matmul(
                                ps[:, half * 512:half * 512 + 512],
                                kT[po:po + 64, ft, t * 128:t * 128 + 128],
                                qT[po:po + 64, ft, :],
                                start=True, stop=True)
                        pt = pt_pool.tile([128, S], BF16, name="pT",
                                          tag="pT")
                        nc.scalar.activation(out=pt, in_=ps, func=AF.Exp,
                                             scale=INV_SQRT)
                        pts.append(pt)
                    pctx = ps_ctx.tile([128, 512], F32, name="ps_c",
                                       tag="ps_c")
                    for tp in range(4):
                        for half in range(2):
                            t = 2 * tp + half
                            nc.tensor.matmul(
                                pctx[0:65, :],
                                Vaug[:, t, hh * 65:hh * 65 + 65],
                                pts[tp][:, half * 512:half * 512 + 512],
                                start=(t == 0), stop=(t == NTOK - 1))
                    rsr = rc_pool.tile([1, 512], F32, name="rsr", tag="rc")
                    nc.vector.tensor_copy(out=rsr, in_=pctx[64:65, :])
                    rch = rc_pool.tile([1, 512], F32, name="rch", tag="rc")
                    nc.vector.reciprocal_approx_fast(out=rch, in_=rsr)
                    rb = rc_pool.tile([64, 512], F32, name="rb", tag="rc")
                    nc.gpsimd.partition_broadcast(rb, rch)
                    nc.vector.tensor_mul(
                        out=ctxT[po:po + 64, ft, :],
                        in0=pctx[0:64, :], in1=rb)

                load_xot()
                with nc.named_scope("p4_kq_v_attn"):
                    for t in range(NTOK):
                        v_proj(t)
                    for jj in range(4):
                        kq_proj(2 * jj)
                        kq_proj(2 * jj + 1)
                        for hh in range(4 * jj, 4 * jj + 4):
                            attn_head(hh)

            if DBG:
                nc.sync.dma_start(out=dbg_kT.ap(),
                                  in_=kT.rearrange("p a b -> p (a b)"))
                nc.sync.dma_start(out=dbg_qT.ap(),
                                  in_=qT.rearrange("p a b -> p (a b)"))
                nc.sync.dma_start(out=dbg_ctx.ap(),
                                  in_=ctxT.rearrange("p a b -> p (a b)"))
                nc.sync.dma_start(out=dbg_vaug.ap(),
                                  in_=Vaug.rearrange("p a b -> p (a b)"))
            dn_stack.close()   # free kT, Vaug, qT
            pm_stack = contextlib.ExitStack()
            pm_pool = pm_stack.enter_context(
                tc.tile_pool(name="pm_pool", bufs=1, side="right"))
            pmT = pm_pool.tile([128, NTOK, SQ], BF16, name="pmT")

            # ============ tail: per-qt outproj / spmm / fuse / LN1 ========
            tb_pool = est.enter_context(tc.tile_pool(name="tb_pool", bufs=1))
            bcb = tb_pool.tile([128, 4, D], F32, name="bcb")
            nc.gpsimd.dma_start(out=bcb, in_=bcb_p.ap())
            b12_bc = bcb[:, 0, :]
            g1_bc = bcb[:, 1, :]
            g2_bc = bcb[:, 2, :]
            be2_bc = bcb[:, 3, :]

            fse = est.enter_context(tc.tile_pool(name="fse", bufs=1))
            x78_stack = contextlib.ExitStack()
            x78 = x78_stack.enter_context(tc.tile_pool(name="x78", bufs=1))
            xhat = x78.tile([128, NQT, D], F32, name="xhat")
            xg = fse.tile([128, NQT, D], F32, name="xg")
            stats = fse.tile([128, NQT, 2, 6], F32, name="stats")
            mv2 = fse.tile([128, NQT, 2], F32, name="mv2")
            sd = fse.tile([128, NQT], F32, name="sd")
            rstd = fse.tile([128, NQT], F32, name="rstd")

            def ln_normalize(x1, qt):
                for half in range(2):
                    nc.vector.bn_stats(
                        out=stats[:, qt, half, :],
                        in_=x1[:, half * 512:half * 512 + 512])
                nc.vector.bn_aggr(out=mv2[:, qt, :], in_=stats[:, qt])
                nc.scalar.activation(out=sd[:, qt:qt + 1],
                                     in_=mv2[:, qt, 1:2], func=AF.Sqrt,
                                     bias=eps_t, scale=1.0)
                nc.vector.reciprocal(rstd[:, qt:qt + 1], sd[:, qt:qt + 1])
                nc.vector.tensor_scalar(out=x1, in0=x1,
                                        scalar1=mv2[:, qt, 0:1],
                                        scalar2=rstd[:, qt:qt + 1],
                                        op0=OP.subtract, op1=OP.mult)

            with contextlib.ExitStack() as ph5:
                ps_mm = ph5.enter_context(
                    tc.tile_pool(name="ps_mm", bufs=4, space="PSUM"))
                ps_tr = ph5.enter_context(
                    tc.tile_pool(name="ps_tr", bufs=3, space="PSUM"))
                with nc.named_scope("p5_tail"):
                    for qt in range(NQT):
                        # pmT transposes (batched 4 per bank)
                        for tg in range(2):
                            pst4 = ps_tr.tile([128, 512], BF16, name="pstb",
                                              tag="pst")
                            for i in range(4):
                                t = 4 * tg + i
                                nc.tensor.transpose(
                                    pst4[:, i * 128:i * 128 + 128],
                                    psp[qt][:, t * 128:t * 128 + 128],
                                    ident_b)
                            nc.scalar.copy(
                                out=pmT[:, 4 * tg:4 * tg + 4,
                                        qt * 128:qt * 128 + 128],
                                in_=pst4.rearrange("p (a b) -> p a b",
                                                   b=128))
                    for qt in range(NQT):
                        # out_proj for this qt; sig*dense + xot in one STT
                        x1 = xhat[:, qt, :]
                        for nh in range(2):
                            ps = ps_mm.tile([128, 512], F32, name="ps_o",
                                            tag="ps_o")
                            for kc in range(KC):
                                nc.tensor.matmul(
                                    ps, ctxT[:, kc, qt * 128:qt * 128 + 128],
                                    wo_t[:, kc, nh * 512:nh * 512 + 512],
                                    start=(kc == 0), stop=(kc == KC - 1))
                            nc.vector.scalar_tensor_tensor(
                                out=x1[:, nh * 512:nh * 512 + 512],
                                in0=ps, scalar=sig_bc,
                                in1=xot[:, qt, nh * 512:nh * 512 + 512],
                                op0=OP.mult, op1=OP.add)
                        # spmm for this qt, fused with rcp + accumulate
                        for nh in range(2):
                            ps = ps_mm.tile([128, 512], F32, name="ps_o",
                                            tag="ps_o")
                            for t in range(NTOK):
                                nc.tensor.matmul(
                                    ps, pmT[:, t, qt * 128:qt * 128 + 128],
                                    Vsp[:, t, nh * 512:nh * 512 + 512],
                                    start=(t == 0), stop=(t == NTOK - 1))
                            nc.vector.scalar_tensor_tensor(
                                out=x1[:, nh * 512:nh * 512 + 512],
                                in0=ps, scalar=rcp_sp[:, qt:qt + 1],
                                in1=x1[:, nh * 512:nh * 512 + 512],
                                op0=OP.mult, op1=OP.add)
                        if DBG:
                            nc.sync.dma_start(
                                out=dbg_fuse.ap()[qt * 128:qt * 128 + 128],
                                in_=x1)
                        ln_normalize(x1, qt)
                        # transpose xhat -> xlnT (g1/be1 folded into ff1
                        # weights host-side), batched 4 chunks per bank
                        for fg in range(2):
                            pst4 = ps_tr.tile([128, 512], F32, name="pstf",
                                              tag="pst")
                            for i in range(4):
                                fc = 4 * fg + i
                                nc.tensor.transpose(
                                    pst4[:, i * 128:i * 128 + 128],
                                    x1[:, fc * 128:fc * 128 + 128],
                                    ident_f)
                            nc.scalar.copy(
                                out=xlnT[:, 4 * fg:4 * fg + 4,
                                         qt * 128:qt * 128 + 128],
                                in_=pst4.rearrange("p (a b) -> p a b",
                                                   b=128))
                    # xg for LN2 residual -- off the critical path, runs
                    # under ff1
                    for qt in range(NQT):
                        nc.gpsimd.tensor_mul(xg[:, qt, :], xhat[:, qt, :],
                                             g1_bc)
                        nc.gpsimd.tensor_add(xg[:, qt, :], xg[:, qt, :],
                                             b12_bc)
            pm_stack.close()
            wo_stack.close()
            ctx_stack.close()
            psp_stack.close()
            x78_stack.close()

            # ============ ff1 + relu ============
            ff_stack = contextlib.ExitStack()
            h1_pool = ff_stack.enter_context(
                tc.tile_pool(name="h1_pool", bufs=1))
            h1T = h1_pool.tile([128, NF2, SQ], BF16, name="h1T")
            if not zb:
                nc.vector.memset(h1T[:, FC, :], 0.0)
                nc.vector.memset(h1T[0:1, FC, :], 1.0)
            w3str = ff_stack.enter_context(tc.tile_pool(name="w3str",
                                                        bufs=3))
            w4str = ff_stack.enter_context(tc.tile_pool(name="w4str",
                                                        bufs=6))
            ps_f1 = ff_stack.enter_context(
                tc.tile_pool(name="ps_f1", bufs=4, space="PSUM"))
            ps_f2 = ff_stack.enter_context(
                tc.tile_pool(name="ps_f2", bufs=4, space="PSUM"))
            qeng = [nc.scalar, nc.sync, nc.gpsimd]
            with nc.named_scope("p9_ff1"):
                for c in range(8):
                    wt = w3str.tile([128, KC, 512], BF16, name="w1t",
                                    tag="w3")
                    # split each 1MB chunk across two queues
                    e0 = qeng[(2 * c) % 3]
                    e1 = qeng[(2 * c + 1) % 3]
                    e0.dma_start(out=wt[:, 0:4, :], in_=f1.ap()[c][:, 0:4, :])
                    e1.dma_start(out=wt[:, 4:8, :], in_=f1.ap()[c][:, 4:8, :])
                    for dd in range(4):
                        dft = c * 4 + dd
                        ps = ps_f1.tile([128, 512], F32, name="ps_f",
                                        tag="ps_f")
                        for kc in range(KC):
                            nc.tensor.matmul(
                                ps, wt[:, kc, dd * 128:dd * 128 + 128],
                                xlnT[:, kc, :],
                                start=(kc == 0), stop=(kc == KC - 1))
                        nc.scalar.activation(
                            out=h1T[:, dft, :], in_=ps, func=AF.Relu,
                            bias=b1_c[:, dft:dft + 1], scale=1.0)
            xln_stack.close()

            # ============ ff2 (two nh-groups) + residual + LN2 + out ======
            ff_s = fse.tile([128, NQT, D], F32, name="ff_s")
            def ln2_final(qt):
                x2 = ff_s[:, qt, :]
                nc.vector.tensor_scalar(out=x2, in0=x2,
                                        scalar1=mv2[:, qt, 0:1],
                                        scalar2=rstd[:, qt:qt + 1],
                                        op0=OP.subtract, op1=OP.mult)
                ot = fse.tile([128, D], F32, name="out_t",
                              tag="out_t", bufs=2)
                nc.vector.tensor_mul(ot, x2, g2_bc)
                nc.vector.tensor_add(ot, ot, be2_bc)
                nc.sync.dma_start(
                    out=out.ap()[qt * 128:qt * 128 + 128, :], in_=ot)

            with nc.named_scope("p10_ff2"):
                for nh in range(2):
                    pss = [ps_f2.tile([128, 512], F32, name="ps_g",
                                      tag="ps_g") for _ in range(NQT)]
                    for fc in range(NF2):
                        f2c = w4str.tile([128, 512], BF16, name="f2c",
                                         tag="w4")
                        eng = qeng[fc % 3]
                        eng.dma_start(
                            out=f2c,
                            in_=f2.ap()[fc][:, nh * 512:nh * 512 + 512])
                        for qt in range(NQT):
                            nc.tensor.matmul(
                                pss[qt],
                                h1T[:, fc, qt * 128:qt * 128 + 128],
                                f2c, start=(fc == 0),
                                stop=(fc == NF2 - 1))
                    for qt in range(NQT):
                        x2h = ff_s[:, qt, nh * 512:nh * 512 + 512]
                        nc.vector.tensor_add(
                            x2h, pss[qt],
                            xg[:, qt, nh * 512:nh * 512 + 512])
                        nc.vector.bn_stats(out=stats[:, qt, nh, :], in_=x2h)
                for qt in range(NQT):
                    nc.vector.bn_aggr(out=mv2[:, qt, :], in_=stats[:, qt])
                nc.scalar.activation(
                    out=sd, in_=mv2.rearrange("p a b -> p (a b)")[:, 1::2],
                    func=AF.Sqrt, bias=eps_t, scale=1.0)
                nc.vector.reciprocal(rstd, sd)
                for qt in range(NQT):
                    ln2_final(qt)
            ff_stack.close()

    nc.compile()
    return nc


def _prep_inputs(src, in_proj_w, in_proj_b, out_proj_w, out_proj_b,
                 Qp_w, Qp_b, Kp_w, Kp_b, Vp_w, Vp_b, lam,
                 ff1_w, ff1_b, ff2_w, ff2_b, ln1_g, ln1_b, ln2_g, ln2_b):
    import ml_dtypes
    f = np.float32
    bf = ml_dtypes.bfloat16
    A = lambda x: np.ascontiguousarray(np.asarray(x), dtype=f)
    AB = lambda x: np.ascontiguousarray(np.asarray(x, dtype=f), dtype=bf)

    W = np.asarray(in_proj_w, dtype=f)
    wq_h = AB(W[0:D].reshape(NFT, 128, KC, 128).transpose(0, 3, 2, 1))
    wk_h = AB(W[D:2 * D].reshape(NFT, 128, KC, 128).transpose(0, 3, 2, 1))

    def aug_moving(wT, bias):
        # [p, kc, f] with an extra chunk whose row 0 is the bias row
        base = np.asarray(wT, dtype=f).reshape(D, KC, 128).transpose(2, 1, 0)
        ext = np.zeros((128, 1, D), f)
        ext[0, 0, :] = np.asarray(bias, f)
        return AB(np.concatenate([base, ext], axis=1))

    wv_h = aug_moving(W[2 * D:3 * D], np.asarray(in_proj_b, f)[2 * D:3 * D])
    vp_h = aug_moving(Vp_w, Vp_b)
    wo_h = AB(np.asarray(out_proj_w, dtype=f).reshape(D, KC, 128)
              .transpose(2, 1, 0))
    qp_h = np.asarray(Qp_w, dtype=f).reshape(R, KC, 128).transpose(2, 1, 0)
    kp_h = np.asarray(Kp_w, dtype=f).reshape(R, KC, 128).transpose(2, 1, 0)
    qkp_h = AB(np.concatenate([qp_h, kp_h], axis=2))
    # fold ln1 affine into ff1: relu((x*g1+be1) @ W1.T + b1)
    #   = relu(x @ (W1*g1).T + (b1 + W1 @ be1))
    W1 = np.asarray(ff1_w, dtype=f)
    W1g = W1 * np.asarray(ln1_g, f)[None, :]
    b1f = np.asarray(ff1_b, f) + W1 @ np.asarray(ln1_b, f)
    f1_h = AB(W1g.reshape(8, 512, KC, 128).transpose(0, 3, 2, 1))
    f2_base = np.asarray(ff2_w, dtype=f).reshape(D, FC, 128) \
        .transpose(1, 2, 0)
    f2_ext = np.zeros((1, 128, D), f)
    f2_ext[0, 0, :] = np.asarray(ff2_b, f)
    f2_h = AB(np.concatenate([f2_base, f2_ext], axis=0))

    bias_cols = np.zeros((128, 80), f)
    bias_cols[:, 0:24] = np.asarray(in_proj_b, f).reshape(24, 128).T
    bias_cols[:, 24:32] = np.asarray(Vp_b, f).reshape(8, 128).T
    bias_cols[:, 32:64] = b1f.reshape(32, 128).T
    bias_cols[:, 64:72] = np.asarray(ln1_g, f).reshape(8, 128).T
    bias_cols[:, 72:80] = np.asarray(ln1_b, f).reshape(8, 128).T
    bqkp_h = np.stack([np.asarray(Qp_b, f), np.asarray(Kp_b, f)], axis=1)
    bqkp_h = np.ascontiguousarray(bqkp_h, f)

    sig = 1.0 / (1.0 + np.exp(-np.float32(np.asarray(lam))))
    bca_h = np.ascontiguousarray(
        np.broadcast_to(sig * np.asarray(out_proj_b, f)[None, :], (128, D)),
        f)
    bcb_h = np.empty((128, 4, D), f)
    bcb_h[:, 0, :] = np.asarray(ln1_b, f)[None, :]
    bcb_h[:, 1, :] = np.asarray(ln1_g, f)[None, :]
    bcb_h[:, 2, :] = np.asarray(ln2_g, f)[None, :]
    bcb_h[:, 3, :] = np.asarray(ln2_b, f)[None, :]
    sig_h = np.full((128, 1), sig, f)
    oms_h = np.full((128, 1), 1.0 - sig, f)

    shared = {
        "wq": wq_h, "wk": wk_h, "wv": wv_h, "vp": vp_h, "wo": wo_h,
        "qkp": qkp_h, "f1": f1_h, "f2": f2_h,
        "bias_cols": bias_cols, "bqkp": bqkp_h,
        "bca": bca_h, "bcb": bcb_h, "sig_col": sig_h, "oms_col": oms_h,
    }
    in_maps = []
    for core in range(8):
        b, h = core // 2, core % 2
        srcb = np.asarray(src[b], dtype=f)
        xTb = srcb.T
        if h == 1:
            # own-query columns first (key order is irrelevant to attention)
            xTb = np.concatenate([xTb[:, SQ:], xTb[:, :SQ]], axis=1)
        m = dict(shared)
        xt_full = np.zeros((KC + 1, 128, S), np.float32)
        xt_full[0:KC] = xTb.reshape(KC, 128, S)
        xt_full[KC, 0, :] = 1.0
        m["xT"] = AB(xt_full)
        m["x_own"] = A(srcb[h * SQ:(h + 1) * SQ, :])
        in_maps.append(m)
    return in_maps


def _zb(inputs):
    z = lambda a: not np.any(np.asarray(a))
    return (z(np.asarray(inputs["in_proj_b"])[2 * D:3 * D])
            and z(inputs["Vp_b"]) and z(inputs["ff2_b"])
            and z(inputs["out_proj_b"]))


def _run(inputs, trace=False):
    zb = _zb(inputs)
    key = ("nc", zb)
    if key not in _cached:
        _cached[key] = _build(zb)
    nc = _cached[key]
    in_maps = _prep_inputs(**inputs)
    res = run_bass_kernel_spmd(nc, in_maps, core_ids=list(range(8)),
                               trace=trace)
    out = np.empty((B, S, D), np.float32)
    for core in range(8):
        b, h = core // 2, core % 2
        out[b, h * SQ:(h + 1) * SQ, :] = res.results[core]["out"]
    return out, res


def kernel(**inputs) -> np.ndarray:
    out, _ = _run(inputs, trace=False)
    return out


# revision 52
# speedup vs baseline: 1.1563x; 1.1563x over previous
"""Trainium2 Bass kernel for the EnhancedEncoderLayer (dense MHA + low-rank
top-k sparse attention + FFN, two layernorms).

Sharding: 8 cores = (batch b in 0..3) x (query-half h in {0,1}). Each core
computes output rows [b, h*512:(h+1)*512, :]. K/V-side projections are
computed redundantly per batch pair (no cross-core communication).

The host permutes src[b].T columns so each core's own query tokens are
columns 0..511 (attention contracts over all keys, so key order is
irrelevant); this keeps the SPMD program identical across cores.

v2 highlights vs the f32r baseline:
- whole trunk in bf16 (weights host-prepped into contiguous stream-order
  layouts -> trivial DMA descriptors, half the HBM traffic);
- v / v-sparse projections are x-stationary and written token-major
  directly (no PE transposes, no ACT copies);
- softmax exps processed as 2-PSUM-bank [128,1024] ACTs;
- top-k threshold bisection runs on bf16 scores, 13 iters, counts split
  between DVE (exact is_ge, qt 0,1) and the ACT engine (sign-sum, qt 2,3);
- ln1 gamma/beta folded into ff1 weights host-side; v/ff2 biases folded
  into the matmuls via augmented ones-row contraction chunks;
- tail is qt-pipelined: out_proj/spmm/fuse/LN1/xhat-transpose per query
  tile; ff2 runs nh-grouped with partial LN2 stats.
"""
import sys
import os
import contextlib

for _p in ('/opt/trn_rl_repo',):
    if _p not in sys.path:
        sys.path.insert(0, _p)

import numpy as np
import concourse.bacc as bacc
import concourse.tile as tile
from concourse import mybir
from concourse.bass_utils import run_bass_kernel_spmd
from concourse.masks import make_identity

F32 = mybir.dt.float32
BF16 = mybir.dt.bfloat16
AF = mybir.ActivationFunctionType
OP = mybir.AluOpType

B, S, D, H, R, DFF = 4, 1024, 1024, 16, 64, 4096
DH = D // H          # 64
SQ = S // 2          # 512 own queries per core
KK = max(1, int(S * 0.2))   # 204
KC = D // 128        # 8 contraction chunks over D
FC = DFF // 128      # 32 chunks over DFF
NQT = SQ // 128      # 4 query tiles
NTOK = S // 128      # 8 token tiles
NFT = KC             # 8 feature tiles of 128 over D
BISECT_ITERS = 13
INV_SQRT = 0.125     # 1/sqrt(DH) == 1/sqrt(R)

_cached = {}


def _build(zb=False):
    # zb: v/vsp, ff2 and out_proj biases are all exactly zero -- skip the
    # augmented bias chunks and the xot bias add entirely.
    nc = bacc.Bacc()
    NKV = KC if zb else KC + 1
    NF2 = FC if zb else FC + 1

    def din(name, shape, dt=F32):
        return nc.declare_dram_parameter(name, list(shape), dt, isOutput=False)

    xT = din("xT", [KC + 1, 128, S], BF16)   # [kc, p, s]; kc=8: ones row
    x_own = din("x_own", [SQ, D])            # own rows, token-major, f32
    wq = din("wq", [NFT, 128, KC, 128], BF16)
    wk = din("wk", [NFT, 128, KC, 128], BF16)
    wv = din("wv", [128, KC + 1, D], BF16)   # kc=8 row0: bias
    vp = din("vp", [128, KC + 1, D], BF16)
    wo = din("wo", [128, KC, D], BF16)
    qkp = din("qkp", [128, KC, 2 * R], BF16)  # cols 0:64 Qp, 64:128 Kp
    f1 = din("f1", [8, 128, KC, 512], BF16)
    f2 = din("f2", [FC + 1, 128, D], BF16)  # chunk FC row0: b2
    bias_cols = din("bias_cols", [128, 80])  # 0:24 qkv, 24:32 vp, 32:64 ff1,
    #                                          64:72 ln1_g, 72:80 ln1_b
    bqkp = din("bqkp", [64, 2])              # col0 Qp_b, col1 Kp_b
    # host-broadcast rows: a = (sig*bo, vb, vspb); b = (b2+be1, g1, g2, be2)
    bca_p = din("bca", [128, D])
    bcb_p = din("bcb", [128, 4, D])
    sig_col = din("sig_col", [128, 1])
    oms_col = din("oms_col", [128, 1])
    out = nc.declare_dram_parameter("out", [SQ, D], F32, isOutput=True)
    DBG = bool(os.environ.get("BASSK_DEBUG"))
    if DBG:
        dbg_fuse = nc.declare_dram_parameter("dbg_fuse", [SQ, D], F32,
                                             isOutput=True)
        dbg_lo = nc.declare_dram_parameter("dbg_lo", [128, NQT], F32,
                                           isOutput=True)
        dbg_rs = nc.declare_dram_parameter("dbg_rs", [128, NQT], F32,
                                           isOutput=True)
        dbg_psp = nc.declare_dram_parameter("dbg_psp", [128, S], BF16,
                                            isOutput=True)
        dbg_kT = nc.declare_dram_parameter("dbg_kT", [128, KC * S], BF16,
                                           isOutput=True)
        dbg_qT = nc.declare_dram_parameter("dbg_qT", [128, KC * SQ], BF16,
                                           isOutput=True)
        dbg_ctx = nc.declare_dram_parameter("dbg_ctx", [128, KC * SQ], BF16,
                                            isOutput=True)
        dbg_vaug = nc.declare_dram_parameter("dbg_vaug",
                                             [128, NTOK * H * (DH + 1)],
                                             BF16, isOutput=True)

    with tile.TileContext(nc) as tc:
        est = contextlib.ExitStack()
        with est:
            # ---------------- constants ----------------
            consts = est.enter_context(tc.tile_pool(name="consts", bufs=1))

            ident_f = consts.tile([128, 128], F32, name="ident_f")
            make_identity(nc, ident_f)
            ident_b = consts.tile([128, 128], BF16, name="ident_b")
            nc.vector.tensor_copy(out=ident_b, in_=ident_f)

            eps_t = consts.tile([128, 1], F32, name="eps_t")
            nc.vector.memset(eps_t, 1e-5)
            ones_b = consts.tile([128, 1], BF16, name="ones_b")
            nc.vector.memset(ones_b, 1.0)
            ones1 = consts.tile([128, 1], F32, name="ones1")
            nc.vector.memset(ones1, 1.0)
            ones16 = consts.tile([128, 16], BF16, name="ones16")
            nc.vector.memset(ones16, 1.0)

            sig_bc = consts.tile([128, 1], F32, name="sig_bc")
            nc.gpsimd.dma_start(out=sig_bc, in_=sig_col.ap())
            oms_bc = consts.tile([128, 1], F32, name="oms_bc")
            nc.gpsimd.dma_start(out=oms_bc, in_=oms_col.ap())

            bc = consts.tile([128, 80], F32, name="bc")
            nc.gpsimd.dma_start(out=bc, in_=bias_cols.ap())
            bqkv_c = bc[:, 0:24]
            b1_c = bc[:, 32:64]
            g1_c = bc[:, 64:72]
            be1_c = bc[:, 72:80]
            bqkp_t = consts.tile([64, 2], F32, name="bqkp_t")
            nc.gpsimd.dma_start(out=bqkp_t, in_=bqkp.ap())
            bqp_c = bqkp_t[:, 0:1]
            bkp_c = bqkp_t[:, 1:2]

            # host-broadcast rows needed during the attention window
            bo_sig = consts.tile([128, D], F32, name="bo_sig")
            nc.gpsimd.dma_start(out=bo_sig, in_=bca_p.ap())

            # own-token residual (+ sig*bo) -- loaded later, used in tail
            xot_pool = est.enter_context(tc.tile_pool(name="xot_pool",
                                                      bufs=1))
            xot = xot_pool.tile([128, NQT, D], F32, name="xot")

            def load_xot():
                for qt in range(NQT):
                    nc.gpsimd.dma_start(
                        out=xot[:, qt, :],
                        in_=x_own.ap()[qt * 128:qt * 128 + 128, :])
                    if not zb:
                        nc.gpsimd.tensor_add(xot[:, qt, :], xot[:, qt, :],
                                             bo_sig)

            # bisect state
            bis = est.enter_context(tc.tile_pool(name="bis", bufs=1))
            lo = bis.tile([128, NQT], F32, name="lo")
            hi = bis.tile([128, NQT], F32, name="hi")
            mid = bis.tile([128, NQT], F32, name="mid")
            cnts = bis.tile([128, NQT], F32, name="cnts")
            pred = bis.tile([128, NQT], mybir.dt.uint32, name="pred")
            rs_sp = bis.tile([128, NQT], F32, name="rs_sp")
            rcp_sp = bis.tile([128, NQT], F32, name="rcp_sp")
            scr_d = bis.tile([128, S], BF16, name="scr_d")
            scr_p = bis.tile([128, S], BF16, name="scr_p")
            nmid = bis.tile([128, NQT], F32, name="nmid")
            thr = bis.tile([128, NQT], F32, name="thr")
            # exact counts for qt 0,1 (DVE is_ge); sign-sum counts for qt 2,3
            # (ACT): sum(sign(x - mid)) >= 2*KK - S  <=>  count(>=mid) >= KK
            nc.vector.memset(thr[:, 0:2], float(KK))
            nc.vector.memset(thr[:, 2:4], float(2 * KK - S))

            # long-lived activation tiles
            sp_pool = est.enter_context(tc.tile_pool(name="sp_pool", bufs=1))
            Vsp = sp_pool.tile([128, NTOK, D], BF16, name="Vsp")  # tok-major
            kspT = sp_pool.tile([64, S], BF16, name="kspT")
            qspT = sp_pool.tile([64, SQ], BF16, name="qspT")

            dn_stack = contextlib.ExitStack()
            dn_pool = dn_stack.enter_context(
                tc.tile_pool(name="dn_pool", bufs=1))
            kT = dn_pool.tile([128, KC, S], BF16, name="kT")
            Vaug = dn_pool.tile([128, NTOK, H * (DH + 1)], BF16, name="Vaug")
            qT = dn_pool.tile([128, KC, SQ], BF16, name="qT")

            Vaug_h = Vaug.rearrange("p t (h c) -> p t h c", c=DH + 1)
            for t in range(NTOK):
                nc.gpsimd.tensor_copy(out=Vaug_h[:, t, :, DH:DH + 1],
                                      in_=ones16)

            xln_stack = contextlib.ExitStack()
            xln_pool = xln_stack.enter_context(
                tc.tile_pool(name="xln_pool", bufs=1, side="right"))
            xlnT = xln_pool.tile([128, KC, SQ], BF16, name="xlnT")

            psp_stack = contextlib.ExitStack()
            psp_pool = psp_stack.enter_context(
                tc.tile_pool(name="psp_pool", bufs=1, side="right"))
            psp = [psp_pool.tile([128, S], BF16, name=f"psp{qt}")
                   for qt in range(NQT)]
            ctx_stack = contextlib.ExitStack()
            ctx_pool = ctx_stack.enter_context(
                tc.tile_pool(name="ctx_pool", bufs=1, side="right"))
            ctxT = ctx_pool.tile([128, KC, SQ], BF16, name="ctxT")

            # out_proj weights (resident through the tail)
            wo_stack = contextlib.ExitStack()
            wo_pool = wo_stack.enter_context(
                tc.tile_pool(name="wo_pool", bufs=1, side="right"))
            wo_t = wo_pool.tile([128, KC, D], BF16, name="wo_t")

            # ============ projections + sparse path + attention ============
            with contextlib.ExitStack() as ph0:
                xt_pool = ph0.enter_context(
                    tc.tile_pool(name="xt_pool", bufs=1))
                wres = ph0.enter_context(tc.tile_pool(name="wres", bufs=1))
                wstr = ph0.enter_context(tc.tile_pool(name="wstr", bufs=3))
                pt_pool = ph0.enter_context(
                    tc.tile_pool(name="pt_pool", bufs=6))
                rc_pool = ph0.enter_context(
                    tc.tile_pool(name="rc_pool", bufs=3))
                ps_small = ph0.enter_context(
                    tc.tile_pool(name="ps_small", bufs=2, space="PSUM"))
                ps_big = ph0.enter_context(
                    tc.tile_pool(name="ps_big", bufs=2, space="PSUM"))
                ps_ctx = ph0.enter_context(
                    tc.tile_pool(name="ps_ctx", bufs=2, space="PSUM"))

                # small sparse weights first, then xT on both queues
                qkpt = wstr.tile([128, KC, 2 * R], BF16, name="qkpt",
                                 tag="wt")
                nc.sync.dma_start(out=qkpt, in_=qkp.ap())
                qeng = [nc.scalar, nc.sync, nc.gpsimd]
                xTt = xt_pool.tile([128, KC + 1, S], BF16, name="xTt")
                wv_h = wres.tile([128, KC + 1, D], BF16, name="wv_t",
                                 tag="wv")
                vp_h = wres.tile([128, KC + 1, D], BF16, name="vp_t",
                                 tag="vp")
                for kc in range(NKV):
                    eng = nc.scalar if kc % 2 == 0 else nc.sync
                    eng.dma_start(out=xTt[:, kc, :], in_=xT.ap()[kc])
                for kc in range(NKV):
                    qeng[(kc + 1) % 3].dma_start(out=wv_h[:, kc, :],
                                                 in_=wv.ap()[:, kc, :])
                    qeng[(kc + 2) % 3].dma_start(out=vp_h[:, kc, :],
                                                 in_=vp.ap()[:, kc, :])
                nc.gpsimd.dma_start(out=wo_t[:, 0:4, :],
                                    in_=wo.ap()[:, 0:4, :])
                nc.gpsimd.dma_start(out=wo_t[:, 4:8, :],
                                    in_=wo.ap()[:, 4:8, :])

                # ---- sparse projections ----
                qpt = qkpt[:, :, 0:R]
                kpt = qkpt[:, :, R:2 * R]
                with nc.named_scope("p0_ksp_qsp"):
                    for nh in range(2):
                        ps = ps_small.tile([128, 512], F32, name="ps",
                                           tag="ps")
                        for kc in range(KC):
                            nc.tensor.matmul(
                                ps[0:64, :], kpt[:, kc, :],
                                xTt[:, kc, nh * 512:nh * 512 + 512],
                                start=(kc == 0), stop=(kc == KC - 1))
                        nc.vector.tensor_scalar(
                            out=kspT[:, nh * 512:nh * 512 + 512],
                            in0=ps[0:64, :], scalar1=bkp_c, scalar2=None,
                            op0=OP.add)
                    ps = ps_small.tile([128, 512], F32, name="ps", tag="ps")
                    for kc in range(KC):
                        nc.tensor.matmul(ps[0:64, :], qpt[:, kc, :],
                                         xTt[:, kc, 0:SQ],
                                         start=(kc == 0), stop=(kc == KC - 1))
                    nc.vector.tensor_scalar(
                        out=qspT, in0=ps[0:64, :], scalar1=bqp_c,
                        scalar2=None, op0=OP.add)

                with nc.named_scope("p2_ssp"):
                    for qt in range(NQT):
                        ps = ps_big.tile([128, S], F32, name="ps2", tag="ps2")
                        for nh in range(2):
                            nc.tensor.matmul(
                                ps[:, nh * 512:nh * 512 + 512],
                                qspT[:, qt * 128:qt * 128 + 128],
                                kspT[:, nh * 512:nh * 512 + 512],
                                start=True, stop=True)
                        nc.scalar.activation(out=psp[qt], in_=ps,
                                             func=AF.Exp, scale=INV_SQRT)

                # ---- top-k threshold bisection (DVE qt 0,1 / Pool qt 2,3) --
                with nc.named_scope("p3_bisect"):
                    nc.vector.memset(lo, 0.0)
                    nc.vector.memset(hi, 16.0)
                    for it in range(BISECT_ITERS):
                        nc.vector.tensor_add(mid, lo, hi)
                        nc.vector.tensor_scalar_mul(mid, mid, 0.5)
                        nc.vector.tensor_scalar_mul(nmid, mid, -1.0)
                        for qt in range(2):
                            nc.vector.scalar_tensor_tensor(
                                out=scr_d, in0=psp[qt],
                                scalar=mid[:, qt:qt + 1],
                                in1=ones_b.to_broadcast([128, S]),
                                op0=OP.is_ge, op1=OP.mult,
                                accum_out=cnts[:, qt:qt + 1])
                        for qt in range(2, NQT):
                            nc.scalar.activation(
                                out=scr_p, in_=psp[qt], func=AF.Sign,
                                bias=nmid[:, qt:qt + 1], scale=1.0,
                                accum_out=cnts[:, qt:qt + 1])
                        nc.vector.tensor_tensor(out=pred, in0=cnts, in1=thr,
                                                op=OP.is_ge)
                        nc.vector.copy_predicated(lo, pred, mid)
                        nc.vector.tensor_tensor(out=pred, in0=cnts, in1=thr,
                                                op=OP.is_lt)
                        nc.vector.copy_predicated(hi, pred, mid)
                    for qt in range(NQT):
                        eng = nc.vector
                        eng.scalar_tensor_tensor(
                            out=psp[qt], in0=psp[qt],
                            scalar=lo[:, qt:qt + 1],
                            in1=psp[qt], op0=OP.is_ge, op1=OP.mult,
                            accum_out=rs_sp[:, qt:qt + 1])
                    if DBG:
                        nc.sync.dma_start(out=dbg_lo.ap(), in_=lo)
                        nc.sync.dma_start(out=dbg_rs.ap(), in_=rs_sp)
                        nc.sync.dma_start(out=dbg_psp.ap(), in_=psp[0])
                    nc.vector.tensor_scalar(out=rs_sp, in0=rs_sp,
                                            scalar1=1e-9, scalar2=None,
                                            op0=OP.add)
                    nc.vector.reciprocal(rcp_sp, rs_sp)
                    nc.vector.tensor_scalar_mul(rcp_sp, rcp_sp, oms_bc)

                # ---- interleaved kq proj + v/vsp proj + dense attention ----
                def kq_proj(ft):
                    wkc = wstr.tile([128, KC, 128], BF16, name="wt",
                                    tag="wt")
                    eng = nc.scalar if ft % 2 == 0 else nc.sync
                    eng.dma_start(out=wkc, in_=wk.ap()[ft])
                    for nh in range(2):
                        ps = ps_small.tile([128, 512], F32, name="ps",
                                           tag="ps")
                        for kc in range(KC):
                            nc.tensor.matmul(
                                ps, wkc[:, kc, :],
                                xTt[:, kc, nh * 512:nh * 512 + 512],
                                start=(kc == 0), stop=(kc == KC - 1))
                        nc.vector.tensor_scalar(
                            out=kT[:, ft, nh * 512:nh * 512 + 512],
                            in0=ps, scalar1=bqkv_c[:, 8 + ft:8 + ft + 1],
                            scalar2=None, op0=OP.add)
                    wqc = wstr.tile([128, KC, 128], BF16, name="wt",
                                    tag="wt")
                    eng.dma_start(out=wqc, in_=wq.ap()[ft])
                    ps = ps_small.tile([128, 512], F32, name="ps", tag="ps")
                    for kc in range(KC):
                        nc.tensor.matmul(ps, wqc[:, kc, :],
                                         xTt[:, kc, 0:SQ],
                                         start=(kc == 0), stop=(kc == KC - 1))
                    nc.vector.tensor_scalar(
                        out=qT[:, ft, :], in0=ps,
                        scalar1=bqkv_c[:, ft:ft + 1], scalar2=None,
                        op0=OP.add)

                def v_proj(t):
                    # x-stationary: out token-major [tok128, D]; bias folded
                    # into the 9th contraction chunk (ones row x bias row)
                    for path in range(2):  # 0: dense v -> Vaug, 1: vsp -> Vsp
                        w_h = wv_h if path == 0 else vp_h
                        psA = ps_small.tile([128, 512], F32, name="ps",
                                            tag="ps")
                        psB = ps_small.tile([128, 512], F32, name="ps",
                                            tag="ps")
                        for kc in range(NKV):
                            lhs = xTt[:, kc, t * 128:t * 128 + 128]
                            nc.tensor.matmul(psA, lhs, w_h[:, kc, 0:512],
                                             start=(kc == 0),
                                             stop=(kc == NKV - 1))
                            nc.tensor.matmul(psB, lhs, w_h[:, kc, 512:D],
                                             start=(kc == 0),
                                             stop=(kc == NKV - 1))
                        if path == 0:
                            pa = psA.rearrange("p (h c) -> p h c", c=DH)
                            pb = psB.rearrange("p (h c) -> p h c", c=DH)
                            nc.scalar.copy(
                                out=Vaug_h[:, t, 0:8, 0:DH], in_=pa)
                            nc.scalar.copy(
                                out=Vaug_h[:, t, 8:16, 0:DH], in_=pb)
                        else:
                            nc.scalar.copy(out=Vsp[:, t, 0:512], in_=psA)
                            nc.scalar.copy(out=Vsp[:, t, 512:D], in_=psB)

                def attn_head(hh):
                    po = 64 * (hh % 2)
                    ft = hh // 2
                    pts = []
                    for tp in range(4):
                        ps = ps_big.tile([128, S], F32, name="ps_s",
                                         tag="ps2")
                        for half in range(2):
                            t = 2 * tp + half
                            nc.tensor.matmul(
                                ps[:, half * 512:half * 512 + 512],
                                kT[po:po + 64, ft, t * 128:t * 128 + 128],
                                qT[po:po + 64, ft, :],
                                start=True, stop=True)
                        pt = pt_pool.tile([128, S], BF16, name="pT",
                                          tag="pT")
                        nc.scalar.activation(out=pt, in_=ps, func=AF.Exp,
                                             scale=INV_SQRT)
                        pts.append(pt)
                    pctx = ps_ctx.tile([128, 512], F32, name="ps_c",
                                       tag="ps_c")
                    for tp in range(4):
                        for half in range(2):
                            t = 2 * tp + half
                            nc.tensor.matmul(
                                pctx[0:65, :],
                                Vaug[:, t, hh * 65:hh * 65 + 65],
                                pts[tp][:, half * 512:half * 512 + 512],
                                start=(t == 0), stop=(t == NTOK - 1))
                    rsr = rc_pool.tile([1, 512], F32, name="rsr", tag="rc")
                    nc.vector.tensor_copy(out=rsr, in_=pctx[64:65, :])
                    rch = rc_pool.tile([1, 512], F32, name="rch", tag="rc")
                    nc.vector.reciprocal_approx_fast(out=rch, in_=rsr)
                    rb = rc_pool.tile([64, 512], F32, name="rb", tag="rc")
                    nc.gpsimd.partition_broadcast(rb, rch)
                    nc.vector.tensor_mul(
                        out=ctxT[po:po + 64, ft, :],
                        in0=pctx[0:64, :], in1=rb)

                load_xot()
                with nc.named_scope("p4_kq_v_attn"):
                    for t in range(NTOK):
                        v_proj(t)
                    for jj in range(4):
                        kq_proj(2 * jj)
                        kq_proj(2 * jj + 1)
                        for hh in range(4 * jj, 4 * jj + 4):
                            attn_head(hh)

            if DBG:
                nc.sync.dma_start(out=dbg_kT.ap(),
                                  in_=kT.rearrange("p a b -> p (a b)"))
                nc.sync.dma_start(out=dbg_qT.ap(),
                                  in_=qT.rearrange("p a b -> p (a b)"))
                nc.sync.dma_start(out=dbg_ctx.ap(),
                                  in_=ctxT.rearrange("p a b -> p (a b)"))
                nc.sync.dma_start(out=dbg_vaug.ap(),
                                  in_=Vaug.rearrange("p a b -> p (a b)"))
            dn_stack.close()   # free kT, Vaug, qT
            pm_stack = contextlib.ExitStack()
            pm_pool = pm_stack.enter_context(
                tc.tile_pool(name="pm_pool", bufs=1, side="right"))
            pmT = pm_pool.tile([128, NTOK, SQ], BF16, name="pmT")

            # ============ tail: per-qt outproj / spmm / fuse / LN1 ========
            tb_pool = est.enter_context(tc.tile_pool(name="tb_pool", bufs=1))
            bcb = tb_pool.tile([128, 4, D], F32, name="bcb")
            nc.gpsimd.dma_start(out=bcb, in_=bcb_p.ap())
            b12_bc = bcb[:, 0, :]
            g1_bc = bcb[:, 1, :]
            g2_bc = bcb[:, 2, :]
            be2_bc = bcb[:, 3, :]

            fse = est.enter_context(tc.tile_pool(name="fse", bufs=1))
            x78_stack = contextlib.ExitStack()
            x78 = x78_stack.enter_context(tc.tile_pool(name="x78", bufs=1))
            xhat = x78.tile([128, NQT, D], F32, name="xhat")
            xg = fse.tile([128, NQT, D], F32, name="xg")
            stats = fse.tile([128, NQT, 2, 6], F32, name="stats")
            mv2 = fse.tile([128, NQT, 2], F32, name="mv2")
            sd = fse.tile([128, NQT], F32, name="sd")
            rstd = fse.tile([128, NQT], F32, name="rstd")

            def ln_normalize(x1, qt):
                for half in range(2):
                    nc.vector.bn_stats(
                        out=stats[:, qt, half, :],
                        in_=x1[:, half * 512:half * 512 + 512])
                nc.vector.bn_aggr(out=mv2[:, qt, :], in_=stats[:, qt])
                nc.scalar.activation(out=sd[:, qt:qt + 1],
                                     in_=mv2[:, qt, 1:2], func=AF.Sqrt,
                                     bias=eps_t, scale=1.0)
                nc.vector.reciprocal(rstd[:, qt:qt + 1], sd[:, qt:qt + 1])
                nc.vector.tensor_scalar(out=x1, in0=x1,
                                        scalar1=mv2[:, qt, 0:1],
                                        scalar2=rstd[:, qt:qt + 1],
                                        op0=OP.subtract, op1=OP.mult)

            with contextlib.ExitStack() as ph5:
                ps_mm = ph5.enter_context(
                    tc.tile_pool(name="ps_mm", bufs=4, space="PSUM"))
                ps_tr = ph5.enter_context(
                    tc.tile_pool(name="ps_tr", bufs=3, space="PSUM"))
                with nc.named_scope("p5_tail"):
                    for qt in range(NQT):
                        # pmT transposes (batched 4 per bank)
                        for tg in range(2):
                            pst4 = ps_tr.tile([128, 512], BF16, name="pstb",
                                              tag="pst")
                            for i in range(4):
                                t = 4 * tg + i
                                nc.tensor.transpose(
                                    pst4[:, i * 128:i * 128 + 128],
                                    psp[qt][:, t * 128:t * 128 + 128],
                                    ident_b)
                            nc.scalar.copy(
                                out=pmT[:, 4 * tg:4 * tg + 4,
                                        qt * 128:qt * 128 + 128],
                                in_=pst4.rearrange("p (a b) -> p a b",
                                                   b=128))
                    for qt in range(NQT):
                        # out_proj for this qt; sig*dense + xot in one STT
                        x1 = xhat[:, qt, :]
                        for nh in range(2):
                            ps = ps_mm.tile([128, 512], F32, name="ps_o",
                                            tag="ps_o")
                            for kc in range(KC):
                                nc.tensor.matmul(
                                    ps, ctxT[:, kc, qt * 128:qt * 128 + 128],
                                    wo_t[:, kc, nh * 512:nh * 512 + 512],
                                    start=(kc == 0), stop=(kc == KC - 1))
                            nc.vector.scalar_tensor_tensor(
                                out=x1[:, nh * 512:nh * 512 + 512],
                                in0=ps, scalar=sig_bc,
                                in1=xot[:, qt, nh * 512:nh * 512 + 512],
                                op0=OP.mult, op1=OP.add)
                        # spmm for this qt, fused with rcp + accumulate
                        for nh in range(2):
                            ps = ps_mm.tile([128, 512], F32, name="ps_o",
                                            tag="ps_o")
                            for t in range(NTOK):
                                nc.tensor.matmul(
                                    ps, pmT[:, t, qt * 128:qt * 128 + 128],
                                    Vsp[:, t, nh * 512:nh * 512 + 512],
                                    start=(t == 0), stop=(t == NTOK - 1))
                            nc.vector.scalar_tensor_tensor(
                                out=x1[:, nh * 512:nh * 512 + 512],
                                in0=ps, scalar=rcp_sp[:, qt:qt + 1],
                                in1=x1[:, nh * 512:nh * 512 + 512],
                                op0=OP.mult, op1=OP.add)
                        if DBG:
                            nc.sync.dma_start(
                                out=dbg_fuse.ap()[qt * 128:qt * 128 + 128],
                                in_=x1)
                        ln_normalize(x1, qt)
                        # transpose xhat -> xlnT (g1/be1 folded into ff1
                        # weights host-side), batched 4 chunks per bank
                        for fg in range(2):
                            pst4 = ps_tr.tile([128, 512], F32, name="pstf",
                                              tag="pst")
                            for i in range(4):
                                fc = 4 * fg + i
                                nc.tensor.transpose(
                                    pst4[:, i * 128:i * 128 + 128],
                                    x1[:, fc * 128:fc * 128 + 128],
                                    ident_f)
                            nc.scalar.copy(
                                out=xlnT[:, 4 * fg:4 * fg + 4,
                                         qt * 128:qt * 128 + 128],
                                in_=pst4.rearrange("p (a b) -> p a b",
                                                   b=128))
                    # xg for LN2 residual -- off the critical path, runs
                    # under ff1
                    for qt in range(NQT):
                        nc.gpsimd.tensor_mul(xg[:, qt, :], xhat[:, qt, :],
                                             g1_bc)
                        nc.gpsimd.tensor_add(xg[:, qt, :], xg[:, qt, :],
                                             b12_bc)
            pm_stack.close()
            wo_stack.close()
            ctx_stack.close()
            psp_stack.close()
            x78_stack.close()

            # ============ ff1 + relu ============
            ff_stack = contextlib.ExitStack()
            h1_pool = ff_stack.enter_context(
                tc.tile_pool(name="h1_pool", bufs=1))
            h1T = h1_pool.tile([128, NF2, SQ], BF16, name="h1T")
            if not zb:
                nc.vector.memset(h1T[:, FC, :], 0.0)
                nc.vector.memset(h1T[0:1, FC, :], 1.0)
            w3str = ff_stack.enter_context(tc.tile_pool(name="w3str",
                                                        bufs=3))
            w4str = ff_stack.enter_context(tc.tile_pool(name="w4str",
                                                        bufs=6))
            ps_f1 = ff_stack.enter_context(
                tc.tile_pool(name="ps_f1", bufs=4, space="PSUM"))
            ps_f2 = ff_stack.enter_context(
                tc.tile_pool(name="ps_f2", bufs=4, space="PSUM"))
            qeng = [nc.scalar, nc.sync, nc.gpsimd]
            with nc.named_scope("p9_ff1"):
                for c in range(8):
                    wt = w3str.tile([128, KC, 512], BF16, name="w1t",
                                    tag="w3")
                    # split each 1MB chunk across two queues
                    e0 = qeng[(2 * c) % 3]
                    e1 = qeng[(2 * c + 1) % 3]
                    e0.dma_start(out=wt[:, 0:4, :], in_=f1.ap()[c][:, 0:4, :])
                    e1.dma_start(out=wt[:, 4:8, :], in_=f1.ap()[c][:, 4:8, :])
                    for dd in range(4):
                        dft = c * 4 + dd
                        ps = ps_f1.tile([128, 512], F32, name="ps_f",
                                        tag="ps_f")
                        for kc in range(KC):
                            nc.tensor.matmul(
                                ps, wt[:, kc, dd * 128:dd * 128 + 128],
                                xlnT[:, kc, :],
                                start=(kc == 0), stop=(kc == KC - 1))
                        nc.scalar.activation(
                            out=h1T[:, dft, :], in_=ps, func=AF.Relu,
                            bias=b1_c[:, dft:dft + 1], scale=1.0)
            xln_stack.close()

            # ============ ff2 (two nh-groups) + residual + LN2 + out ======
            ff_s = fse.tile([128, NQT, D], F32, name="ff_s")
            def ln2_final(qt):
                x2 = ff_s[:, qt, :]
                nc.vector.tensor_scalar(out=x2, in0=x2,
                                        scalar1=mv2[:, qt, 0:1],
                                        scalar2=rstd[:, qt:qt + 1],
                                        op0=OP.subtract, op1=OP.mult)
                ot = fse.tile([128, D], F32, name="out_t",
                              tag="out_t", bufs=2)
                nc.vector.tensor_mul(ot, x2, g2_bc)
                nc.vector.tensor_add(ot, ot, be2_bc)
                nc.sync.dma_start(
                    out=out.ap()[qt * 128:qt * 128 + 128, :], in_=ot)

            with nc.named_scope("p10_ff2"):
                for nh in range(2):
                    pss = [ps_f2.tile([128, 512], F32, name="ps_g",
                                      tag="ps_g") for _ in range(NQT)]
                    for fc in range(NF2):
                        f2c = w4str.tile([128, 512], BF16, name="f2c",
                                         tag="w4")
                        eng = qeng[fc % 3]
                        eng.dma_start(
                            out=f2c,
                            in_=f2.ap()[fc][:, nh * 512:nh * 512 + 512])
                        for qt in range(NQT):
                            nc.tensor.matmul(
                                pss[qt],
                                h1T[:, fc, qt * 128:qt * 128 + 128],
                                f2c, start=(fc == 0),
                                stop=(fc == NF2 - 1))
                    for qt in range(NQT):
                        x2h = ff_s[:, qt, nh * 512:nh * 512 + 512]
                        nc.vector.tensor_add(
                            x2h, pss[qt],
                            xg[:, qt, nh * 512:nh * 512 + 512])
                        nc.vector.bn_stats(out=stats[:, qt, nh, :], in_=x2h)
                for qt in range(NQT):
                    nc.vector.bn_aggr(out=mv2[:, qt, :], in_=stats[:, qt])
                nc.scalar.activation(
                    out=sd, in_=mv2.rearrange("p a b -> p (a b)")[:, 1::2],
                    func=AF.Sqrt, bias=eps_t, scale=1.0)
                nc.vector.reciprocal(rstd, sd)
                for qt in range(NQT):
                    ln2_final(qt)
            ff_stack.close()

    nc.compile()
    return nc


def _prep_inputs(src, in_proj_w, in_proj_b, out_proj_w, out_proj_b,
                 Qp_w, Qp_b, Kp_w, Kp_b, Vp_w, Vp_b, lam,
                 ff1_w, ff1_b, ff2_w, ff2_b, ln1_g, ln1_b, ln2_g, ln2_b):
    import ml_dtypes
    f = np.float32
    bf = ml_dtypes.bfloat16
    A = lambda x: np.ascontiguousarray(np.asarray(x), dtype=f)
    AB = lambda x: np.ascontiguousarray(np.asarray(x, dtype=f), dtype=bf)

    W = np.asarray(in_proj_w, dtype=f)
    wq_h = AB(W[0:D].reshape(NFT, 128, KC, 128).transpose(0, 3, 2, 1))
    wk_h = AB(W[D:2 * D].reshape(NFT, 128, KC, 128).transpose(0, 3, 2, 1))

    def aug_moving(wT, bias):
        # [p, kc, f] with an extra chunk whose row 0 is the bias row
        base = np.asarray(wT, dtype=f).reshape(D, KC, 128).transpose(2, 1, 0)
        ext = np.zeros((128, 1, D), f)
        ext[0, 0, :] = np.asarray(bias, f)
        return AB(np.concatenate([base, ext], axis=1))

    wv_h = aug_moving(W[2 * D:3 * D], np.asarray(in_proj_b, f)[2 * D:3 * D])
    vp_h = aug_moving(Vp_w, Vp_b)
    wo_h = AB(np.asarray(out_proj_w, dtype=f).reshape(D, KC, 128)
              .transpose(2, 1, 0))
    qp_h = np.asarray(Qp_w, dtype=f).reshape(R, KC, 128).transpose(2, 1, 0)
    kp_h = np.asarray(Kp_w, dtype=f).reshape(R, KC, 128).transpose(2, 1, 0)
    qkp_h = AB(np.concatenate([qp_h, kp_h], axis=2))
    # fold ln1 affine into ff1: relu((x*g1+be1) @ W1.T + b1)
    #   = relu(x @ (W1*g1).T + (b1 + W1 @ be1))
    W1 = np.asarray(ff1_w, dtype=f)
    W1g = W1 * np.asarray(ln1_g, f)[None, :]
    b1f = np.asarray(ff1_b, f) + W1 @ np.asarray(ln1_b, f)
    f1_h = AB(W1g.reshape(8, 512, KC, 128).transpose(0, 3, 2, 1))
    f2_base = np.asarray(ff2_w, dtype=f).reshape(D, FC, 128) \
        .transpose(1, 2, 0)
    f2_ext = np.zeros((1, 128, D), f)
    f2_ext[0, 0, :] = np.asarray(ff2_b, f)
    f2_h = AB(np.concatenate([f2_base, f2_ext], axis=0))

    bias_cols = np.zeros((128, 80), f)
    bias_cols[:, 0:24] = np.asarray(in_proj_b, f).reshape(24, 128).T
    bias_cols[:, 24:32] = np.asarray(Vp_b, f).reshape(8, 128).T
    bias_cols[:, 32:64] = b1f.reshape(32, 128).T
    bias_cols[:, 64:72] = np.asarray(ln1_g, f).reshape(8, 128).T
    bias_cols[:, 72:80] = np.asarray(ln1_b, f).reshape(8, 128).T
    bqkp_h = np.stack([np.asarray(Qp_b, f), np.asarray(Kp_b, f)], axis=1)
    bqkp_h = np.ascontiguousarray(bqkp_h, f)

    sig = 1.0 / (1.0 + np.exp(-np.float32(np.asarray(lam))))
    bca_h = np.ascontiguousarray(
        np.broadcast_to(sig * np.asarray(out_proj_b, f)[None, :], (128, D)),
        f)
    bcb_h = np.empty((128, 4, D), f)
    bcb_h[:, 0, :] = np.asarray(ln1_b, f)[None, :]
    bcb_h[:, 1, :] = np.asarray(ln1_g, f)[None, :]
    bcb_h[:, 2, :] = np.asarray(ln2_g, f)[None, :]
    bcb_h[:, 3, :] = np.asarray(ln2_b, f)[None, :]
    sig_h = np.full((128, 1), sig, f)
    oms_h = np.full((128, 1), 1.0 - sig, f)

    shared = {
        "wq": wq_h, "wk": wk_h, "wv": wv_h, "vp": vp_h, "wo": wo_h,
        "qkp": qkp_h, "f1": f1_h, "f2": f2_h,
        "bias_cols": bias_cols, "bqkp": bqkp_h,
        "bca": bca_h, "bcb": bcb_h, "sig_col": sig_h, "oms_col": oms_h,
    }
    in_maps = []
    for core in range(8):
        b, h = core // 2, core % 2
        srcb = np.asarray(src[b], dtype=f)
        xTb = srcb.T
        if h == 1:
            # own-query columns first (key order is irrelevant to attention)
            xTb = np.concatenate([xTb[:, SQ:], xTb[:, :SQ]], axis=1)
        m = dict(shared)
        xt_full = np.zeros((KC + 1, 128, S), np.float32)
        xt_full[0:KC] = xTb.reshape(KC, 128, S)
        xt_full[KC, 0, :] = 1.0
        m["xT"] = AB(xt_full)
        m["x_own"] = A(srcb[h * SQ:(h + 1) * SQ, :])
        in_maps.append(m)
    return in_maps


def _zb(inputs):
    z = lambda a: not np.any(np.asarray(a))
    return (z(np.asarray(inputs["in_proj_b"])[2 * D:3 * D])
            and z(inputs["Vp_b"]) and z(inputs["ff2_b"])
            and z(inputs["out_proj_b"]))


def _run(inputs, trace=False):
    zb = False  # zb variant measured slower on hw; keep the general path
    key = ("nc", zb)
    if key not in _cached:
        _cached[key] = _build(zb)
    nc = _cached[key]
    in_maps = _prep_inputs(**inputs)
    res = run_bass_kernel_spmd(nc, in_maps, core_ids=list(range(8)),
                               trace=trace)
    out = np.empty((B, S, D), np.float32)
    for core in range(8):
        b, h = core // 2, core % 2
        out[b, h * SQ:(h + 1) * SQ, :] = res.results[core]["out"]
    return out, res


def kernel(**inputs) -> np.ndarray:
    out, _ = _run(inputs, trace=False)
    return out
